# revision 2
# baseline (speedup 1.0000x reference)
"""Causal self-attention (B=2, T=2048, C=1024, H=16, RoPE) on 8 trn2 cores.

v2: bf16 matmul inputs everywhere (fp32 PSUM accumulation), fine-grained
software-pipelined issue order.

Sharding: core c = 4*b + g handles batch b and head group g (4 heads):
column-parallel W_qkv, local attention, row-parallel W_proj with the
all-reduce done on host (partial [T, C] outputs summed in numpy).

Layout: q,k kept transposed [d, t] (bf16); scores computed transposed
[tk, tq] in fp32 PSUM; exp (scale folded) evicts PSUM->SBUF as bf16
probs; PV accumulates yT [65, 512] (ones column appended to v gives row
sums); normalization multiplies by a gpsimd-broadcast 1/sum.

Issue order interleaves three streams (qkv+rope, attention, projection)
at ~1us bundles so the in-order PE queue always holds independent work
while exps/evictions complete: attention bundles alternate with qkv/proj
bundles dispensed proportionally to remaining work.

Engines: PE matmuls; Act exp + v evictions; DVE rope muls/add, recip,
normalize, out evictions (PSUM readers); Pool triu masking + partition
broadcast (no PSUM port).
"""
import numpy as np
import ml_dtypes
import concourse.bass as bass
import concourse.mybir as mybir
import concourse.tile as tile
from concourse import bacc
from concourse.bass import ts, ds
from concourse.bass_utils import run_bass_kernel_spmd
from contextlib import ExitStack
from collections import deque

F32 = mybir.dt.float32
BF16 = mybir.dt.bfloat16
EXP = mybir.ActivationFunctionType.Exp
NPBF = ml_dtypes.bfloat16

B, T, C, H, DH = 2, 2048, 1024, 16, 64
NCORE, G = 8, 4
HPG = H // G             # heads per group = 4
CT = C // 128            # 8 c-tiles
QC = T // 512            # 4 query chunks
SCALE = 1.0 / np.sqrt(DH)
ROPE_BASE = 10000.0


def _build_nc(reps=1):
    nc = bacc.Bacc("TRN2", target_bir_lowering=False, debug=False)

    # host-pretiled layouts: XH[p, qc, ct, tc] = x[b].T[128*ct+p, 512*qc+tc]
    XH = nc.dram_tensor("XH", [128, QC * CT * 512], BF16, kind="ExternalInput")
    WQKH = nc.dram_tensor("WQKH", [128, CT * 512], BF16, kind="ExternalInput")
    WVH = nc.dram_tensor("WVH", [128, CT * 256], BF16, kind="ExternalInput")
    WP = nc.dram_tensor("Wp", [256, C], BF16, kind="ExternalInput")
    COS2 = nc.dram_tensor("COS2", [128, T], BF16, kind="ExternalInput")
    S2P = nc.dram_tensor("S2P", [128, T], BF16, kind="ExternalInput")
    PI = nc.dram_tensor("PI", [128, 128], BF16, kind="ExternalInput")
    TRIU = nc.dram_tensor("TRIU", [128, 128], BF16, kind="ExternalInput")
    OUT = nc.dram_tensor("out", [T, C], BF16, kind="ExternalOutput")

    with tile.TileContext(nc) as tc, ExitStack() as top:
        const = top.enter_context(tc.tile_pool(name="const", bufs=1))
        pi_sb = const.tile([128, 128], BF16, tag="pi", name="pi_sb")
        triu_sb = const.tile([128, 128], BF16, tag="triu", name="triu_sb")
        nc.sync.dma_start(out=pi_sb[:], in_=PI[:])
        nc.sync.dma_start(out=triu_sb[:], in_=TRIU[:])

        persist = top.enter_context(tc.tile_pool(name="persist", bufs=1))
        qkT = [persist.tile([128, T], BF16, tag=f"qkT{j}", name=f"qkT{j}") for j in range(4)]
        v_sb = persist.tile([128, 4 * QC, HPG, DH + 1], BF16, tag="v", name="v_sb")
        nc.gpsimd.memset(v_sb[:, :, :, DH:DH + 1], 1.0)
        yTn = [persist.tile([128, T], BF16, tag=f"yTn{j}", name=f"yTn{j}") for j in range(2)]

        bw = top.enter_context(tc.tile_pool(name="bw", bufs=1))
        xt_all = bw.tile([128, QC, CT, 512], BF16, tag="xt", name="xt_all")
        wqk_all = bw.tile([128, CT, 512], BF16, tag="wqk", name="wqk_all")
        wv_all = bw.tile([128, CT, 256], BF16, tag="wv", name="wv_all")
        wp_sb = [bw.tile([128, C], BF16, tag=f"wp{i}", name=f"wp{i}") for i in range(2)]
        cos_t = bw.tile([128, T], BF16, tag="cos", name="cos_t")
        s2_t = bw.tile([128, T], BF16, tag="s2", name="s2_t")

        # PSUM: scores 2x[128,1024] (4 banks) + yT 2x[65,512] (2) + misc 2x[128,512] (2)
        psS = top.enter_context(tc.tile_pool(name="psS", bufs=2, space="PSUM"))
        psY = top.enter_context(tc.tile_pool(name="psY", bufs=2, space="PSUM"))
        psQ = top.enter_context(tc.tile_pool(name="psQ", bufs=2, space="PSUM"))
        rope_p = top.enter_context(tc.tile_pool(name="rope_p", bufs=2))
        ptp = top.enter_context(tc.tile_pool(name="ptp", bufs=4))
        smal = top.enter_context(tc.tile_pool(name="smal", bufs=2))
        outp = top.enter_context(tc.tile_pool(name="outp", bufs=4))

        def issue_dmas():
            # consumption order; big contiguous granules (HWDGE issue is
            # ~625ns per dma_start, so instruction count gates the intro)
            def xdma(qc, half):
                nc.sync.dma_start(
                    out=xt_all[:, qc, 4 * half:4 * half + 4, :].rearrange("p a b -> p (a b)"),
                    in_=XH[:, ds(qc * CT * 512 + 2048 * half, 2048)],
                )

            nc.sync.dma_start(
                out=wqk_all[:, 0:4, :].rearrange("p a b -> p (a b)"),
                in_=WQKH[:, ds(0, 2048)])
            xdma(0, 0)
            nc.sync.dma_start(out=cos_t[:, 0:1024], in_=COS2[:, 0:1024])
            nc.sync.dma_start(out=s2_t[:, 0:1024], in_=S2P[:, 0:1024])
            nc.sync.dma_start(
                out=wqk_all[:, 4:8, :].rearrange("p a b -> p (a b)"),
                in_=WQKH[:, ds(2048, 2048)])
            xdma(0, 1)
            nc.sync.dma_start(
                out=wv_all[:, :, :].rearrange("p a b -> p (a b)"), in_=WVH[:, :])
            xdma(1, 0)
            xdma(1, 1)
            nc.sync.dma_start(out=cos_t[:, 1024:2048], in_=COS2[:, 1024:2048])
            nc.sync.dma_start(out=s2_t[:, 1024:2048], in_=S2P[:, 1024:2048])
            for i in range(2):
                nc.sync.dma_start(out=wp_sb[i][:], in_=WP[ts(i, 128), :])
            for qc in (2, 3):
                xdma(qc, 0)
                xdma(qc, 1)

        # ---- qkv + rope ----
        # split per jt into two yields so ps_rot (which waits on a DVE mul)
        # can have foreign PE work queued before it
        def qk_part1(qc, jt, st):
            ps = psQ.tile([128, 512], F32, tag="q", name="ps_qk")
            for ct in range(CT):
                nc.tensor.matmul(
                    ps[:], wqk_all[:, ct, ts(jt, 128)], xt_all[:, qc, ct, :],
                    start=(ct == 0), stop=(ct == CT - 1),
                )
            raw2 = rope_p.tile([128, 512], BF16, tag="raw", name="raw2")
            nc.vector.tensor_mul(raw2[:], ps[:], s2_t[:, ts(qc, 512)])
            t1 = rope_p.tile([128, 512], BF16, tag="t1", name="t1")
            nc.vector.tensor_mul(t1[:], ps[:], cos_t[:, ts(qc, 512)])
            st["raw2"], st["t1"] = raw2, t1

        def qk_part2(qc, jt, st):
            ps_rot = psQ.tile([128, 512], F32, tag="q", name="ps_rot")
            nc.tensor.matmul(ps_rot[:], pi_sb[:], st["raw2"][:], start=True, stop=True)
            nc.vector.tensor_add(qkT[jt][:, ts(qc, 512)], st["t1"][:], ps_rot[:])

        def v_bundle(qc, vj):
            for t4 in (2 * vj, 2 * vj + 1):
                tt = 4 * qc + t4
                ps = psQ.tile([128, 512], F32, tag="q", name="ps_v")
                for ct in range(CT):
                    nc.tensor.matmul(
                        ps[:, 0:256], xt_all[:, qc, ct, ts(t4, 128)], wv_all[:, ct, :],
                        start=(ct == 0), stop=(ct == CT - 1),
                    )
                nc.scalar.copy(
                    v_sb[:, tt, :, 0:DH],
                    ps[:, 0:256].rearrange("p (h d) -> p h d", h=HPG),
                )

        def qkv_gen(qc):
            for jt in range(4):
                st = {}
                qk_part1(qc, jt, st)
                yield
                qk_part2(qc, jt, st)
                yield
            for vj in range(2):
                v_bundle(qc, vj)
                yield

        # ---- attention ----
        def grp_bundle(qc, hp, g, yT):
            qT, kT = qkT[hp], qkT[2 + hp]
            ss = []
            for h2 in range(2):
                p0 = 64 * h2
                s = psS.tile([128, 1024], F32, tag="s", name="s_off")
                for j2 in range(2):
                    ktile = 2 * g + j2
                    nc.tensor.matmul(
                        s[:, ts(j2, 512)],
                        kT[p0:p0 + 64, ts(ktile, 128)],
                        qT[p0:p0 + 64, ts(qc, 512)],
                        start=True, stop=True,
                    )
                ss.append(s)
            pts = []
            for h2 in range(2):
                pt = ptp.tile([128, 1024], BF16, tag="pt", name="pt")
                nc.scalar.activation(pt[:], ss[h2][:], EXP, scale=SCALE)
                pts.append(pt)
            for h2 in range(2):
                for j2 in range(2):
                    ktile = 2 * g + j2
                    nc.tensor.matmul(
                        yT[h2][:],
                        v_sb[:, ktile, 2 * hp + h2, :],
                        pts[h2][:, ts(j2, 512)],
                        start=(ktile == 0), stop=False,
                        skip_group_check=True,
                    )

        def diag_bundle(qc, hp, h2, yT):
            qT, kT = qkT[hp], qkT[2 + hp]
            p0 = 64 * h2
            head = 2 * hp + h2
            # j0 (N=512) + j1 (N=384) in a [128,1024] tile; j2 (256) + j3 (128)
            # in a [128,512] tile
            sm = psS.tile([128, 1024], F32, tag="s", name="s_dm")
            stl = psQ.tile([128, 512], F32, tag="q", name="s_dt")
            spans = [(0, 512, sm, 0), (1, 384, sm, 512), (2, 256, stl, 0), (3, 128, stl, 256)]
            for j, n, tile_, off in spans:
                nc.tensor.matmul(
                    tile_[:, ds(off, n)],
                    kT[p0:p0 + 64, ts(4 * qc + j, 128)],
                    qT[p0:p0 + 64, ds(512 * qc + 128 * j, n)],
                    start=True, stop=True,
                )
            ptm = ptp.tile([128, 1024], BF16, tag="pt", name="ptm")
            nc.scalar.activation(ptm[:, 0:896], sm[:, 0:896], EXP, scale=SCALE)
            ptt = ptp.tile([128, 512], BF16, tag="ptt", bufs=2, name="ptt")
            nc.scalar.activation(ptt[:, 0:384], stl[:, 0:384], EXP, scale=SCALE)
            pmap = {0: (ptm, 0), 1: (ptm, 512), 2: (ptt, 0), 3: (ptt, 256)}
            for j, n, _, _ in spans:
                pt, off = pmap[j]
                nc.gpsimd.tensor_mul(pt[:, ds(off, 128)], pt[:, ds(off, 128)], triu_sb[:])
            for j, n, _, _ in spans:
                pt, off = pmap[j]
                nc.tensor.matmul(
                    yT[h2][:, ds(128 * j, n)],
                    v_sb[:, 4 * qc + j, head, :],
                    pt[:, ds(off, n)],
                    start=(qc == 0 and j == 0), stop=(j == 3),
                    skip_group_check=True,
                )

        def fin_part(qc, hp, h2, yT, split=1):
            w = 512 // split
            for piece in range(split):
                sl = ds(piece * w, w)
                r = smal.tile([1, 512], F32, tag="r", bufs=2, name="r")
                nc.vector.reciprocal(r[:, 0:w], yT[h2][DH:DH + 1, sl])
                rbc = smal.tile([64, 512], F32, tag="rbc", bufs=2, name="rbc")
                nc.gpsimd.partition_broadcast(rbc[:, 0:w], r[:, 0:w])
                nc.vector.tensor_mul(
                    yTn[hp][64 * h2:64 * h2 + 64, ds(512 * qc + piece * w, w)],
                    yT[h2][0:DH, sl], rbc[:, 0:w],
                )

        fin_issued = {}

        def attn_gen(qc, hp):
            yT = [psY.tile([DH + 1, 512], F32, tag="y", name=f"yT{h2}")
                  for h2 in range(2)]
            for g in range(2 * qc):
                grp_bundle(qc, hp, g, yT)
                yield
            last = (qc == QC - 1 and hp == 1)
            for h2 in range(2):
                diag_bundle(qc, hp, h2, yT)
                fin_part(qc, hp, h2, yT, split=2 if last else 1)
                yield
            fin_issued[(qc, hp)] = True

        # ---- projection ----
        def proj_bundle(tt):
            ob = outp.tile([128, 1024], BF16, tag="ob", name="ob")
            if tt >= 12:
                # tail: scores pool is idle — use one big tile + whole evict
                pp = psS.tile([128, 1024], F32, tag="s", name="pp_big")
                for half in range(2):
                    for jt in range(2):
                        nc.tensor.matmul(
                            pp[:, ts(half, 512)],
                            yTn[jt][:, ts(tt, 128)],
                            wp_sb[jt][:, ts(half, 512)],
                            start=(jt == 0), stop=(jt == 1),
                        )
                if tt % 2:
                    nc.scalar.copy(ob[:], pp[:])
                else:
                    nc.vector.tensor_copy(ob[:], pp[:])
            else:
                for half in range(2):
                    pp = psQ.tile([128, 512], F32, tag="q", name="pp")
                    for jt in range(2):
                        nc.tensor.matmul(
                            pp[:],
                            yTn[jt][:, ts(tt, 128)],
                            wp_sb[jt][:, ts(half, 512)],
                            start=(jt == 0), stop=(jt == 1),
                        )
                    nc.vector.tensor_copy(ob[:, ts(half, 512)], pp[:])
            nc.sync.dma_start(out=OUT[ts(tt, 128), :], in_=ob[:])

        def proj_gen(tt):
            proj_bundle(tt)
            yield

        # ---- interleaved schedule ----
        def schedule():
            issue_dmas()
            fin_issued.clear()
            # A(0) first — nothing else can run yet
            for _ in qkv_gen(0):
                pass
            qkv_issued = [True, False, False, False]

            others = deque()
            for qc in range(1, QC):
                others.append(["A", qc, qkv_gen(qc), 10])
            for tt in range(4 * QC):
                others.append(["C", tt, proj_gen(tt), 1])
            others_rem = 3 * 10 + 16

            pairs = [(qc, hp) for qc in range(QC) for hp in range(2)]
            attn_rem = sum(2 * qc + 2 for qc, hp in pairs)
            pi_idx = 0
            cur_attn = None
            credit = 0.0

            def step_other():
                nonlocal others_rem
                for i, ent in enumerate(others):
                    kind, key, gen, _ = ent
                    if kind == "C" and not fin_issued.get((key // 4, 1)):
                        continue
                    try:
                        next(gen)
                        ent[3] -= 1
                        others_rem -= 1
                        if ent[3] == 0:
                            if kind == "A":
                                qkv_issued[key] = True
                            del others[i]
                        return True
                    except StopIteration:
                        if kind == "A":
                            qkv_issued[key] = True
                        del others[i]
                        others_rem -= ent[3]
                        return True
                return False

            while attn_rem > 0 or others:
                stepped_attn = False
                if cur_attn is None and pi_idx < len(pairs):
                    qc, hp = pairs[pi_idx]
                    if qkv_issued[qc]:
                        cur_attn = attn_gen(qc, hp)
                if cur_attn is not None:
                    try:
                        next(cur_attn)
                        stepped_attn = True
                    except StopIteration:
                        cur_attn = None
                        pi_idx += 1
                        continue
                    attn_rem -= 1
                if stepped_attn:
                    if attn_rem > 0:
                        credit += others_rem / attn_rem
                        n = int(credit)
                        credit -= n
                    else:
                        n = len(others) + others_rem
                    for _ in range(n):
                        if not step_other():
                            break
                else:
                    if not step_other():
                        # nothing ready at all (shouldn't happen)
                        if cur_attn is None and pi_idx >= len(pairs):
                            break

        for _rep in range(reps):
            schedule()

    nc.finalize()
    return nc


def _rope_tables():
    inv_freq = (1.0 / (ROPE_BASE ** (np.arange(0, DH, 2, dtype=np.float32) / DH))).astype(np.float32)
    t = np.arange(T, dtype=np.float32)
    freqs = np.einsum("i,j->ij", t, inv_freq).astype(np.float32)
    emb = np.concatenate([freqs, freqs], axis=-1)          # [T, DH]
    cosT = np.cos(emb).astype(np.float32).T.copy()         # [64, T]
    sinT = np.sin(emb).astype(np.float32).T.copy()
    sgn = np.ones((DH, 1), dtype=np.float32)
    sgn[0:DH // 2] = -1.0
    s2 = (sgn * sinT).astype(np.float32)
    s2p = np.roll(s2, -32, axis=0)                         # s2p[j] = s2[(j+32)%64]
    cos2 = np.concatenate([cosT, cosT], axis=0)            # [128, T]
    s2p2 = np.concatenate([s2p, s2p], axis=0)
    return np.ascontiguousarray(cos2), np.ascontiguousarray(s2p2)


_NC_CACHE = None
LAST_EXEC_NS = None


def _prepare_in_maps(x, W_qkv, W_proj):
    x = np.asarray(x, dtype=np.float32)
    W_qkv = np.asarray(W_qkv, dtype=np.float32)
    W_proj = np.asarray(W_proj, dtype=np.float32)

    cos2, s2p = _rope_tables()
    cos2 = cos2.astype(NPBF)
    s2p = s2p.astype(NPBF)
    pi = np.zeros((128, 128), dtype=np.float32)
    half = DH // 2
    for blk in range(2):
        for i in range(DH):
            pi[64 * blk + i, 64 * blk + (i + half) % DH] = 1.0
    pi = pi.astype(NPBF)
    triu = np.triu(np.ones((128, 128), dtype=np.float32)).astype(NPBF)

    Wq, Wk, Wv_full = W_qkv[:, 0:C], W_qkv[:, C:2 * C], W_qkv[:, 2 * C:3 * C]

    def ctile(a):
        # [C, F] -> [128, CT*F]   (c-tile-major column blocks)
        f = a.shape[1]
        return np.ascontiguousarray(
            a.reshape(CT, 128, f).transpose(1, 0, 2).reshape(128, CT * f))

    in_maps = []
    for core in range(NCORE):
        b, g = core // G, core % G
        cols = slice(256 * g, 256 * g + 256)
        wqk = np.concatenate([Wq[:, cols], Wk[:, cols]], axis=1)  # [C, 512]
        xT = x[b].T                                               # [C, T]
        # XH[p, (qc, ct, tc)] = xT[128*ct+p, 512*qc+tc]
        xh = (xT.reshape(CT, 128, QC, 512).transpose(1, 2, 0, 3)
              .reshape(128, QC * CT * 512))
        in_maps.append({
            "XH": np.ascontiguousarray(xh).astype(NPBF),
            "WQKH": ctile(wqk).astype(NPBF),
            "WVH": ctile(Wv_full[:, cols]).astype(NPBF),
            "Wp": np.ascontiguousarray(W_proj[cols, :]).astype(NPBF),
            "COS2": cos2, "S2P": s2p, "PI": pi, "TRIU": triu,
        })
    return in_maps


def kernel(x, W_qkv, W_proj):
    global _NC_CACHE
    if _NC_CACHE is None:
        _NC_CACHE = _build_nc()
    nc = _NC_CACHE
    in_maps = _prepare_in_maps(x, W_qkv, W_proj)

    res = run_bass_kernel_spmd(nc, in_maps, list(range(NCORE)))
    global LAST_EXEC_NS
    LAST_EXEC_NS = res.exec_time_ns
    parts = [res.results[i]["out"].astype(np.float64) for i in range(NCORE)]
    out = np.stack([
        parts[0] + parts[1] + parts[2] + parts[3],
        parts[4] + parts[5] + parts[6] + parts[7],
    ])
    return out.astype(np.float32)


# revision 3
# speedup vs baseline: 1.7009x; 1.7009x over previous
"""Causal self-attention (B=2, T=2048, C=1024, H=16, RoPE) on 8 trn2 cores.

v2: bf16 matmul inputs everywhere (fp32 PSUM accumulation), fine-grained
software-pipelined issue order.

Sharding: core c = 4*b + g handles batch b and head group g (4 heads):
column-parallel W_qkv, local attention, row-parallel W_proj with the
all-reduce done on host (partial [T, C] outputs summed in numpy).

Layout: q,k kept transposed [d, t] (bf16); scores computed transposed
[tk, tq] in fp32 PSUM; exp (scale folded) evicts PSUM->SBUF as bf16
probs; PV accumulates yT [65, 512] (ones column appended to v gives row
sums); normalization multiplies by a gpsimd-broadcast 1/sum.

Issue order interleaves three streams (qkv+rope, attention, projection)
at ~1us bundles so the in-order PE queue always holds independent work
while exps/evictions complete: attention bundles alternate with qkv/proj
bundles dispensed proportionally to remaining work.

Engines: PE matmuls; Act exp + v evictions; DVE rope muls/add, recip,
normalize, out evictions (PSUM readers); Pool triu masking + partition
broadcast (no PSUM port).
"""
import numpy as np
import ml_dtypes
import concourse.bass as bass
import concourse.mybir as mybir
import concourse.tile as tile
from concourse import bacc
from concourse.bass import ts, ds
from concourse.bass_utils import run_bass_kernel_spmd
from contextlib import ExitStack
from collections import deque

F32 = mybir.dt.float32
F32R = mybir.dt.float32r
BF16 = mybir.dt.bfloat16
EXP = mybir.ActivationFunctionType.Exp
NPBF = ml_dtypes.bfloat16

B, T, C, H, DH = 2, 2048, 1024, 16, 64
NCORE, G = 8, 4
HPG = H // G             # heads per group = 4
CT = C // 128            # 8 c-tiles
QC = T // 512            # 4 query chunks
SCALE = 1.0 / np.sqrt(DH)
ROPE_BASE = 10000.0


def _build_nc(reps=1):
    nc = bacc.Bacc("TRN2", target_bir_lowering=False, debug=False)

    # host-pretiled layouts: XH[p, qc, ct, tc] = x[b].T[128*ct+p, 512*qc+tc]
    XH = nc.dram_tensor("XH", [128, QC * CT * 512], F32R, kind="ExternalInput")
    WQKH = nc.dram_tensor("WQKH", [128, CT * 512], F32R, kind="ExternalInput")
    WVH = nc.dram_tensor("WVH", [128, CT * 256], F32R, kind="ExternalInput")
    WP = nc.dram_tensor("Wp", [256, C], F32R, kind="ExternalInput")
    COS2 = nc.dram_tensor("COS2", [128, T], F32, kind="ExternalInput")
    S2P = nc.dram_tensor("S2P", [128, T], F32, kind="ExternalInput")
    PI = nc.dram_tensor("PI", [128, 128], F32R, kind="ExternalInput")
    TRIU = nc.dram_tensor("TRIU", [128, 128], F32, kind="ExternalInput")
    VONES = nc.dram_tensor("VONES", [128, 64], F32R, kind="ExternalInput")
    OUT = nc.dram_tensor("out", [T, C], F32, kind="ExternalOutput")

    with tile.TileContext(nc) as tc, ExitStack() as top:
        const = top.enter_context(tc.tile_pool(name="const", bufs=1))
        pi_sb = const.tile([128, 128], F32R, tag="pi", name="pi_sb")
        triu_sb = const.tile([128, 128], F32, tag="triu", name="triu_sb")
        nc.sync.dma_start(out=pi_sb[:], in_=PI[:])
        nc.sync.dma_start(out=triu_sb[:], in_=TRIU[:])

        persist = top.enter_context(tc.tile_pool(name="persist", bufs=1))
        qkT = [persist.tile([128, T], F32R, tag=f"qkT{j}", name=f"qkT{j}") for j in range(4)]
        v_sb = persist.tile([128, 4 * QC, HPG, DH + 1], F32R, tag="v", name="v_sb")
        nc.sync.dma_start(
            out=v_sb[:, :, :, DH:DH + 1].rearrange("p a b c -> p (a b c)"),
            in_=VONES[:],
        )
        yTn = [persist.tile([128, T], F32R, tag=f"yTn{j}", name=f"yTn{j}") for j in range(2)]

        bw = top.enter_context(tc.tile_pool(name="bw", bufs=1))
        # x streams through 2 rotating chunk tiles (each chunk is fully
        # consumed by its qkv phase, so chunk qc+2's DMA waits on qc's reads)
        xtp = top.enter_context(tc.tile_pool(name="xtp", bufs=2))
        wqk_all = bw.tile([128, 4, CT, 128], F32R, tag="wqk", name="wqk_all")
        wv_all = bw.tile([128, CT, 256], F32R, tag="wv", name="wv_all")
        wp_sb = [bw.tile([128, C], F32R, tag=f"wp{i}", name=f"wp{i}") for i in range(2)]
        cos_t = bw.tile([128, T], F32, tag="cos", name="cos_t")
        s2_t = bw.tile([128, T], F32, tag="s2", name="s2_t")

        # PSUM: scores 2x[128,1024] (4 banks) + yT 2x[65,512] (2) + misc 2x[128,512] (2)
        psS = top.enter_context(tc.tile_pool(name="psS", bufs=2, space="PSUM"))
        psY = top.enter_context(tc.tile_pool(name="psY", bufs=2, space="PSUM"))
        psQ = top.enter_context(tc.tile_pool(name="psQ", bufs=2, space="PSUM"))
        rope_p = top.enter_context(tc.tile_pool(name="rope_p", bufs=2))
        ptp = top.enter_context(tc.tile_pool(name="ptp", bufs=3))
        smal = top.enter_context(tc.tile_pool(name="smal", bufs=2))
        outp = top.enter_context(tc.tile_pool(name="outp", bufs=3))

        def issue_dmas():
            # consumption order; big contiguous granules (HWDGE issue is
            # ~625ns per dma_start, so instruction count gates the intro)
            xtile = [None] * QC

            def xdma(qc, half):
                if xtile[qc] is None:
                    xtile[qc] = xtp.tile([128, CT, 512], F32R, tag="xt",
                                         name=f"xt{qc}")
                nc.sync.dma_start(
                    out=xtile[qc][:, 4 * half:4 * half + 4, :].rearrange("p a b -> p (a b)"),
                    in_=XH[:, ds(qc * CT * 512 + 2048 * half, 2048)],
                )

            # WQKH is jt-major: [128, (jt, ct, 128)] so the first qk chain
            # only waits on a 1024-col granule
            def wqkdma(jt):
                nc.sync.dma_start(
                    out=wqk_all[:, jt, :, :].rearrange("p a b -> p (a b)"),
                    in_=WQKH[:, ds(1024 * jt, 1024)])

            wqkdma(0)
            xdma(0, 0)
            xdma(0, 1)
            wqkdma(1)
            nc.sync.dma_start(out=cos_t[:, 0:1024], in_=COS2[:, 0:1024])
            nc.sync.dma_start(out=s2_t[:, 0:1024], in_=S2P[:, 0:1024])
            wqkdma(2)
            wqkdma(3)
            nc.sync.dma_start(
                out=wv_all[:, :, :].rearrange("p a b -> p (a b)"), in_=WVH[:, :])
            xdma(1, 0)
            xdma(1, 1)
            nc.sync.dma_start(out=cos_t[:, 1024:2048], in_=COS2[:, 1024:2048])
            nc.sync.dma_start(out=s2_t[:, 1024:2048], in_=S2P[:, 1024:2048])
            for i in range(2):
                nc.sync.dma_start(out=wp_sb[i][:], in_=WP[ts(i, 128), :])
            for qc in (2, 3):
                xdma(qc, 0)
                xdma(qc, 1)
            return xtile

        # ---- qkv + rope ----
        # split per jt into two yields so ps_rot (which waits on a DVE mul)
        # can have foreign PE work queued before it
        def qk_part1(qc, jt, st, xtile):
            ps = psQ.tile([128, 512], F32, tag="q", name="ps_qk")
            for ct in range(CT):
                nc.tensor.matmul(
                    ps[:], wqk_all[:, jt, ct, :], xtile[qc][:, ct, :],
                    start=(ct == 0), stop=(ct == CT - 1),
                )
            raw2 = rope_p.tile([128, 512], F32R, tag="raw", name="raw2")
            nc.vector.tensor_mul(raw2[:], ps[:], s2_t[:, ts(qc, 512)])
            t1 = rope_p.tile([128, 512], F32, tag="t1", name="t1")
            nc.vector.tensor_mul(t1[:], ps[:], cos_t[:, ts(qc, 512)])
            st["raw2"], st["t1"] = raw2, t1

        def qk_part2(qc, jt, st):
            ps_rot = psQ.tile([128, 512], F32, tag="q", name="ps_rot")
            nc.tensor.matmul(ps_rot[:], pi_sb[:], st["raw2"][:], start=True, stop=True)
            nc.vector.tensor_add(qkT[jt][:, ts(qc, 512)], st["t1"][:], ps_rot[:])

        def v_bundle(qc, vj, xtile):
            for t4 in (2 * vj, 2 * vj + 1):
                tt = 4 * qc + t4
                ps = psQ.tile([128, 512], F32, tag="q", name="ps_v")
                for ct in range(CT):
                    nc.tensor.matmul(
                        ps[:, 0:256], xtile[qc][:, ct, ts(t4, 128)], wv_all[:, ct, :],
                        start=(ct == 0), stop=(ct == CT - 1),
                    )
                nc.scalar.copy(
                    v_sb[:, tt, :, 0:DH],
                    ps[:, 0:256].rearrange("p (h d) -> p h d", h=HPG),
                )

        def qkv_gen(qc, xtile):
            for jt in range(4):
                st = {}
                qk_part1(qc, jt, st, xtile)
                yield
                qk_part2(qc, jt, st)
                yield
            for vj in range(2):
                v_bundle(qc, vj, xtile)
                yield

        # ---- attention ----
        def grp_bundle(qc, hp, g, yT):
            qT, kT = qkT[hp], qkT[2 + hp]
            ss = []
            for h2 in range(2):
                p0 = 64 * h2
                s = psS.tile([128, 1024], F32, tag="s", name="s_off")
                for j2 in range(2):
                    ktile = 2 * g + j2
                    nc.tensor.matmul(
                        s[:, ts(j2, 512)],
                        kT[p0:p0 + 64, ts(ktile, 128)],
                        qT[p0:p0 + 64, ts(qc, 512)],
                        start=True, stop=True,
                    )
                ss.append(s)
            pts = []
            for h2 in range(2):
                pt = ptp.tile([128, 1024], F32R, tag="pt", name="pt")
                nc.scalar.activation(pt[:], ss[h2][:], EXP, scale=SCALE)
                pts.append(pt)
            for h2 in range(2):
                for j2 in range(2):
                    ktile = 2 * g + j2
                    nc.tensor.matmul(
                        yT[h2][:],
                        v_sb[:, ktile, 2 * hp + h2, :],
                        pts[h2][:, ts(j2, 512)],
                        start=(ktile == 0), stop=False,
                        skip_group_check=True,
                    )

        def diag_bundle(qc, hp, h2, yT):
            qT, kT = qkT[hp], qkT[2 + hp]
            p0 = 64 * h2
            head = 2 * hp + h2
            # j0 (N=512) + j1 (N=384) in a [128,1024] tile; j2 (256) + j3 (128)
            # in a [128,512] tile
            sm = psS.tile([128, 1024], F32, tag="s", name="s_dm")
            stl = psQ.tile([128, 512], F32, tag="q", name="s_dt")
            spans = [(0, 512, sm, 0), (1, 384, sm, 512), (2, 256, stl, 0), (3, 128, stl, 256)]
            for j, n, tile_, off in spans:
                w = n
                if w < 256 and qc < QC - 1:
                    w = 256  # fp32r needs N>=256 for full rate; junk cols unread
                nc.tensor.matmul(
                    tile_[:, ds(off, w)],
                    kT[p0:p0 + 64, ts(4 * qc + j, 128)],
                    qT[p0:p0 + 64, ds(512 * qc + 128 * j, w)],
                    start=True, stop=True,
                )
            ptm = ptp.tile([128, 1024], F32R, tag="pt", name="ptm")
            nc.scalar.activation(ptm[:, 0:896], sm[:, 0:896], EXP, scale=SCALE)
            ptt = ptp.tile([128, 512], F32R, tag="ptt", bufs=2, name="ptt")
            nc.scalar.activation(ptt[:, 0:384], stl[:, 0:384], EXP, scale=SCALE)
            pmap = {0: (ptm, 0), 1: (ptm, 512), 2: (ptt, 0), 3: (ptt, 256)}
            for j, n, _, _ in spans:
                pt, off = pmap[j]
                nc.vector.tensor_mul(pt[:, ds(off, 128)], pt[:, ds(off, 128)].bitcast(F32), triu_sb[:])
            for j, n, _, _ in spans:
                pt, off = pmap[j]
                nc.tensor.matmul(
                    yT[h2][:, ds(128 * j, n)],
                    v_sb[:, 4 * qc + j, head, :],
                    pt[:, ds(off, n)],
                    start=(qc == 0 and j == 0), stop=(j == 3),
                    skip_group_check=True,
                )

        def fin_part(qc, hp, h2, yT, split=1):
            w = 512 // split
            for piece in range(split):
                sl = ds(piece * w, w)
                r = smal.tile([1, 512], F32, tag="r", bufs=2, name="r")
                nc.vector.reciprocal(r[:, 0:w], yT[h2][DH:DH + 1, sl])
                rbc = smal.tile([64, 512], F32, tag="rbc", bufs=2, name="rbc")
                nc.gpsimd.partition_broadcast(rbc[:, 0:w], r[:, 0:w])
                nc.vector.tensor_mul(
                    yTn[hp][64 * h2:64 * h2 + 64, ds(512 * qc + piece * w, w)],
                    yT[h2][0:DH, sl], rbc[:, 0:w],
                )

        fin_issued = {}

        def attn_gen(qc, hp):
            yT = [psY.tile([DH + 1, 512], F32, tag="y", name=f"yT{h2}")
                  for h2 in range(2)]
            for g in range(2 * qc):
                grp_bundle(qc, hp, g, yT)
                yield
            last = (qc == QC - 1 and hp == 1)
            for h2 in range(2):
                diag_bundle(qc, hp, h2, yT)
                fin_part(qc, hp, h2, yT, split=2 if last else 1)
                yield
            fin_issued[(qc, hp)] = True

        # ---- projection ----
        def proj_bundle(tt):
            ob = outp.tile([128, 1024], F32, tag="ob", name="ob")
            if tt >= 12:
                # tail: scores pool is idle — use one big tile + whole evict
                pp = psS.tile([128, 1024], F32, tag="s", name="pp_big")
                for half in range(2):
                    for jt in range(2):
                        nc.tensor.matmul(
                            pp[:, ts(half, 512)],
                            yTn[jt][:, ts(tt, 128)],
                            wp_sb[jt][:, ts(half, 512)],
                            start=(jt == 0), stop=(jt == 1),
                        )
                if tt % 2:
                    nc.scalar.copy(ob[:], pp[:])
                else:
                    nc.vector.tensor_copy(ob[:], pp[:])
            else:
                for half in range(2):
                    pp = psQ.tile([128, 512], F32, tag="q", name="pp")
                    for jt in range(2):
                        nc.tensor.matmul(
                            pp[:],
                            yTn[jt][:, ts(tt, 128)],
                            wp_sb[jt][:, ts(half, 512)],
                            start=(jt == 0), stop=(jt == 1),
                        )
                    nc.vector.tensor_copy(ob[:, ts(half, 512)], pp[:])
            nc.sync.dma_start(out=OUT[ts(tt, 128), :], in_=ob[:])

        def proj_gen(tt):
            proj_bundle(tt)
            yield

        # ---- interleaved schedule ----
        def schedule():
            xtile = issue_dmas()
            fin_issued.clear()
            # A(0) first — nothing else can run yet
            for _ in qkv_gen(0, xtile):
                pass
            qkv_issued = [True, False, False, False]

            others = deque()
            for qc in range(1, QC):
                others.append(["A", qc, qkv_gen(qc, xtile), 10])
            for tt in range(4 * QC):
                others.append(["C", tt, proj_gen(tt), 1])
            others_rem = 3 * 10 + 16

            pairs = [(qc, hp) for qc in range(QC) for hp in range(2)]
            attn_rem = sum(2 * qc + 2 for qc, hp in pairs)
            pi_idx = 0
            cur_attn = None
            credit = 0.0

            def step_other():
                nonlocal others_rem
                for i, ent in enumerate(others):
                    kind, key, gen, _ = ent
                    if kind == "C" and not fin_issued.get((key // 4, 1)):
                        continue
                    try:
                        next(gen)
                        ent[3] -= 1
                        others_rem -= 1
                        if ent[3] == 0:
                            if kind == "A":
                                qkv_issued[key] = True
                            del others[i]
                        return True
                    except StopIteration:
                        if kind == "A":
                            qkv_issued[key] = True
                        del others[i]
                        others_rem -= ent[3]
                        return True
                return False

            while attn_rem > 0 or others:
                stepped_attn = False
                if cur_attn is None and pi_idx < len(pairs):
                    qc, hp = pairs[pi_idx]
                    if qkv_issued[qc]:
                        cur_attn = attn_gen(qc, hp)
                if cur_attn is not None:
                    try:
                        next(cur_attn)
                        stepped_attn = True
                    except StopIteration:
                        cur_attn = None
                        pi_idx += 1
                        continue
                    attn_rem -= 1
                if stepped_attn:
                    if attn_rem > 0:
                        credit += others_rem / attn_rem
                        n = int(credit)
                        credit -= n
                    else:
                        n = len(others) + others_rem
                    for _ in range(n):
                        if not step_other():
                            break
                else:
                    if not step_other():
                        # nothing ready at all (shouldn't happen)
                        if cur_attn is None and pi_idx >= len(pairs):
                            break

        for _rep in range(reps):
            schedule()

    nc.finalize()
    return nc


def _rope_tables():
    inv_freq = (1.0 / (ROPE_BASE ** (np.arange(0, DH, 2, dtype=np.float32) / DH))).astype(np.float32)
    t = np.arange(T, dtype=np.float32)
    freqs = np.einsum("i,j->ij", t, inv_freq).astype(np.float32)
    emb = np.concatenate([freqs, freqs], axis=-1)          # [T, DH]
    cosT = np.cos(emb).astype(np.float32).T.copy()         # [64, T]
    sinT = np.sin(emb).astype(np.float32).T.copy()
    sgn = np.ones((DH, 1), dtype=np.float32)
    sgn[0:DH // 2] = -1.0
    s2 = (sgn * sinT).astype(np.float32)
    s2p = np.roll(s2, -32, axis=0)                         # s2p[j] = s2[(j+32)%64]
    cos2 = np.concatenate([cosT, cosT], axis=0)            # [128, T]
    s2p2 = np.concatenate([s2p, s2p], axis=0)
    return np.ascontiguousarray(cos2), np.ascontiguousarray(s2p2)


_NC_CACHE = None
LAST_EXEC_NS = None


def _prepare_in_maps(x, W_qkv, W_proj):
    x = np.asarray(x, dtype=np.float32)
    W_qkv = np.asarray(W_qkv, dtype=np.float32)
    W_proj = np.asarray(W_proj, dtype=np.float32)

    cos2, s2p = _rope_tables()
    pi = np.zeros((128, 128), dtype=np.float32)
    half = DH // 2
    for blk in range(2):
        for i in range(DH):
            pi[64 * blk + i, 64 * blk + (i + half) % DH] = 1.0
        triu = np.triu(np.ones((128, 128), dtype=np.float32))

    Wq, Wk, Wv_full = W_qkv[:, 0:C], W_qkv[:, C:2 * C], W_qkv[:, 2 * C:3 * C]

    def ctile(a):
        # [C, F] -> [128, CT*F]   (c-tile-major column blocks)
        f = a.shape[1]
        return np.ascontiguousarray(
            a.reshape(CT, 128, f).transpose(1, 0, 2).reshape(128, CT * f))

    in_maps = []
    for core in range(NCORE):
        b, g = core // G, core % G
        cols = slice(256 * g, 256 * g + 256)
        wqk = np.concatenate([Wq[:, cols], Wk[:, cols]], axis=1)  # [C, 512]
        # WQKH[p, (jt, ct, c)] = wqk[128*ct+p, 128*jt+c]
        wqkh = (wqk.reshape(CT, 128, 4, 128).transpose(1, 2, 0, 3)
                .reshape(128, 4 * CT * 128))
        xT = x[b].T                                               # [C, T]
        # XH[p, (qc, ct, tc)] = xT[128*ct+p, 512*qc+tc]
        xh = (xT.reshape(CT, 128, QC, 512).transpose(1, 2, 0, 3)
              .reshape(128, QC * CT * 512))
        in_maps.append({
            "XH": np.ascontiguousarray(xh),
            "WQKH": np.ascontiguousarray(wqkh),
            "WVH": ctile(Wv_full[:, cols]),
            "Wp": np.ascontiguousarray(W_proj[cols, :]),
            "COS2": cos2, "S2P": s2p, "PI": pi, "TRIU": triu,
            "VONES": np.ones((128, 64), dtype=np.float32),
        })
    return in_maps


def kernel(x, W_qkv, W_proj):
    global _NC_CACHE
    if _NC_CACHE is None:
        _NC_CACHE = _build_nc()
    nc = _NC_CACHE
    in_maps = _prepare_in_maps(x, W_qkv, W_proj)

    res = run_bass_kernel_spmd(nc, in_maps, list(range(NCORE)))
    global LAST_EXEC_NS
    LAST_EXEC_NS = res.exec_time_ns
    parts = [res.results[i]["out"].astype(np.float64) for i in range(NCORE)]
    out = np.stack([
        parts[0] + parts[1] + parts[2] + parts[3],
        parts[4] + parts[5] + parts[6] + parts[7],
    ])
    return out.astype(np.float32)
